# revision 3
# baseline (speedup 1.0000x reference)
"""Trainium2 Bass kernel for nn_Kongming_SPMM (GNN message passing).

out = V2V@x + V2R@((I+R2R1)(I+R2R0)) R2V@x   with all matrices sparse COO.

Strategy (8 NeuronCores, SPMD single program):
- Destination-row sharding: core k owns rows [k*R/8, (k+1)*R/8) of each
  SpMM's destination space (rules R=20000, nodes R=100000). Host routes
  edges to owner cores, sorts by destination row, packs into 128-edge
  chunks grouped into 128-row PSUM blocks with a *uniform* chunks-per-block
  count C (max over cores/blocks, zero-padded) so one SPMD program fits
  every core; per-core behavior differs only through input data.
- Per chunk on device: indirect-DMA gather of the 128 source rows (bf16,
  edge-major [128e x 64f]) + one fused DVE tensor_scalar builds the
  val-scaled one-hot lhsT [128e x 128r] (iota==rowlocal)*val + one PE
  matmul accumulating into the f32 PSUM block.
- Rule-phase results are AllGathered (DRAM collective) across cores so the
  next phase can gather any rule row; v2v/v2r accumulate into an SBUF
  staging buffer that is DMA'd out once.

Self-contained: only numpy + concourse imports; shapes hardcoded.
"""

import numpy as np
import ml_dtypes

N_NODES = 100000
N_RULES = 20000
D = 64
N_CORES = 8
P = 128
G = 1  # one 128-row indirect gather per chunk (one offset per partition)

_BF16 = ml_dtypes.bfloat16


def _prep_phase(rows, cols, vals, R):
    """Route edges by destination-row owner, sort, pack into uniform
    [128, B*C] streams per core. Returns (offs, rowl, vals, B, C) where
    offs/rowl/vals are lists of per-core arrays [128, B*C]."""
    share = R // N_CORES
    B = (share + P - 1) // P
    rows = np.asarray(rows, dtype=np.int64)
    cols = np.asarray(cols, dtype=np.int64)
    vals = np.asarray(vals, dtype=np.float32)

    owner = rows // share
    per_core = []
    C = 1
    for k in range(N_CORES):
        sel = owner == k
        r = (rows[sel] - k * share).astype(np.int64)
        c = cols[sel]
        v = vals[sel]
        order = np.argsort(r, kind="stable")
        r, c, v = r[order], c[order], v[order]
        block = r >> 7
        counts = np.bincount(block, minlength=B)
        if len(r):
            C = max(C, int((counts.max() + P - 1) // P))
        per_core.append((r, c, v, block, counts))

    offs_l, rowl_l, vals_l = [], [], []
    slots = B * C * P
    for k in range(N_CORES):
        r, c, v, block, counts = per_core[k]
        cum = np.concatenate([[0], np.cumsum(counts)])
        within = np.arange(len(r)) - cum[block]
        pos = block * (C * P) + within
        offs = np.zeros(slots, np.int32)
        rowl = np.zeros(slots, np.float32)
        vv = np.zeros(slots, np.float32)
        offs[pos] = c.astype(np.int32)
        rowl[pos] = (r - block * P).astype(np.float32)
        vv[pos] = v
        nch = B * C
        offs_l.append(offs.reshape(nch, P).T.copy())
        rowl_l.append(rowl.reshape(nch, P).T.copy())
        vals_l.append(vv.reshape(nch, P).T.copy())
    return offs_l, rowl_l, vals_l, B, C


def kernel(**inputs):
    from concourse import bacc, bass, tile
    import concourse.mybir as mybir
    from concourse.bass_utils import run_bass_kernel_spmd

    dt = mybir.dt

    x = np.asarray(inputs["x_j"], np.float32)
    xb = x.astype(_BF16)

    # ---- host prep: 5 phases ----
    # P1: rule0 = R2V @ x
    ph1 = _prep_phase(inputs["r2v_rows"], inputs["r2v_cols"], inputs["r2v_vals"], N_RULES)
    # P2/P3: rule_i+1 = R2R_i @ rule_i + rule_i  -> append identity edges
    ident_r = np.arange(N_RULES, dtype=np.int64)
    ident_v = np.ones(N_RULES, np.float32)
    r2r_rows = np.asarray(inputs["r2r_rows"], np.int64)
    r2r_cols = np.asarray(inputs["r2r_cols"], np.int64)
    r2r_vals = np.asarray(inputs["r2r_vals"], np.float32)
    ph2 = _prep_phase(
        np.concatenate([r2r_rows[0], ident_r]),
        np.concatenate([r2r_cols[0], ident_r]),
        np.concatenate([r2r_vals[0], ident_v]),
        N_RULES,
    )
    ph3 = _prep_phase(
        np.concatenate([r2r_rows[1], ident_r]),
        np.concatenate([r2r_cols[1], ident_r]),
        np.concatenate([r2r_vals[1], ident_v]),
        N_RULES,
    )
    # P4a: out += V2R @ rule2 ; P4b: out += V2V @ x
    ph4 = _prep_phase(inputs["v2r_rows"], inputs["v2r_cols"], inputs["v2r_vals"], N_NODES)
    ph5 = _prep_phase(inputs["v2v_rows"], inputs["v2v_cols"], inputs["v2v_vals"], N_NODES)

    iota_np = np.broadcast_to(np.arange(P, dtype=np.float32), (P, P)).astype(_BF16).copy()

    # ---- build the SPMD program ----
    nc = bacc.Bacc(
        "TRN2",
        target_bir_lowering=False,
        debug=False,
        enable_asserts=False,
        num_devices=N_CORES,
    )
    R_SH = N_RULES // N_CORES      # 2500
    O_SH = N_NODES // N_CORES      # 12500
    OB = (O_SH + P - 1) // P       # 98

    xb_t = nc.dram_tensor("xb", [N_NODES, D], dt.bfloat16, kind="ExternalInput").ap()
    iota_t = nc.dram_tensor("iota", [P, P], dt.bfloat16, kind="ExternalInput").ap()
    out_t = nc.dram_tensor("out_slice", [O_SH, D], dt.float32, kind="ExternalOutput").ap()

    phases = []  # (name, prep, dest_R)
    for name, prep in [("p1", ph1), ("p2", ph2), ("p3", ph3), ("p4", ph4), ("p5", ph5)]:
        offs_l, rowl_l, vals_l, B, C = prep
        nch = B * C
        o_t = nc.dram_tensor(f"{name}_offs", [P, nch], dt.int32, kind="ExternalInput").ap()
        r_t = nc.dram_tensor(f"{name}_rowl", [P, nch], dt.float32, kind="ExternalInput").ap()
        v_t = nc.dram_tensor(f"{name}_vals", [P, nch], dt.float32, kind="ExternalInput").ap()
        phases.append(dict(name=name, B=B, C=C, nch=nch, o=o_t, r=r_t, v=v_t))

    rule_sl = [nc.dram_tensor(f"rule{i}_sl", [R_SH, D], dt.bfloat16) for i in range(3)]
    rule_fl = [
        nc.dram_tensor(f"rule{i}_fl", [N_RULES, D], dt.bfloat16, addr_space="Shared")
        for i in range(3)
    ]

    with tile.TileContext(nc) as tc:
        with (
            tc.tile_pool(name="stream", bufs=1) as spool,
            tc.tile_pool(name="gath", bufs=3) as gpool,
            tc.tile_pool(name="oh", bufs=4) as ohpool,
            tc.tile_pool(name="stage", bufs=2) as stpool,
            tc.tile_pool(name="outb", bufs=1) as obpool,
            tc.tile_pool(name="psum", bufs=4, space="PSUM") as ppool,
        ):
            iota = spool.tile([P, P], dt.bfloat16)
            nc.sync.dma_start(iota[:], iota_t[:])
            outbuf = obpool.tile([P, OB * D], dt.float32)

            def run_phase(ph, src_ap, mode, dst_block_valid, on_block_done):
                B, C, nch = ph["B"], ph["C"], ph["nch"]
                offs = spool.tile([P, nch], dt.int32, name=f"{ph['name']}_o")
                rowl = spool.tile([P, nch], dt.float32, name=f"{ph['name']}_r")
                valt = spool.tile([P, nch], dt.float32, name=f"{ph['name']}_v")
                nc.sync.dma_start(offs[:], ph["o"][:])
                nc.sync.dma_start(rowl[:], ph["r"][:])
                nc.sync.dma_start(valt[:], ph["v"][:])
                pt = None
                for j0 in range(0, nch, G):
                    g = min(G, nch - j0)
                    gt = gpool.tile([P, G * D], dt.bfloat16, tag="gt")
                    nc.gpsimd.indirect_dma_start(
                        out=gt[:, : g * D],
                        out_offset=None,
                        in_=src_ap,
                        in_offset=bass.IndirectOffsetOnAxis(
                            ap=offs[:, j0 : j0 + g], axis=0
                        ),
                    )
                    for cc in range(g):
                        j = j0 + cc
                        b, cj = j // C, j % C
                        oh = ohpool.tile([P, P], dt.bfloat16, tag="oh")
                        nc.vector.tensor_scalar(
                            oh[:],
                            iota[:],
                            rowl[:, j : j + 1],
                            valt[:, j : j + 1],
                            mybir.AluOpType.is_equal,
                            mybir.AluOpType.mult,
                        )
                        if cj == 0:
                            pt = ppool.tile([P, D], dt.float32, tag="acc")
                        nc.tensor.matmul(
                            out=pt[:],
                            lhsT=oh[:],
                            rhs=gt[:, cc * D : (cc + 1) * D],
                            start=(cj == 0),
                            stop=(cj == C - 1),
                        )
                        if cj == C - 1:
                            on_block_done(b, pt)

            # --- rule phases ---
            def make_rule_done(slot):
                def done(b, pt):
                    valid = min(P, R_SH - b * P)
                    st = stpool.tile([P, D], dt.bfloat16, tag="rst")
                    nc.vector.tensor_copy(st[:], pt[:])
                    nc.sync.dma_start(
                        rule_sl[slot][b * P : b * P + valid, :], st[:valid, :]
                    )
                return done

            run_phase(phases[0], xb_t[:], "rule", R_SH, make_rule_done(0))
            nc.gpsimd.collective_compute(
                "AllGather",
                mybir.AluOpType.bypass,
                replica_groups=[list(range(N_CORES))],
                ins=[rule_sl[0][:]],
                outs=[rule_fl[0][:]],
            )
            run_phase(phases[1], rule_fl[0][:], "rule", R_SH, make_rule_done(1))
            nc.gpsimd.collective_compute(
                "AllGather",
                mybir.AluOpType.bypass,
                replica_groups=[list(range(N_CORES))],
                ins=[rule_sl[1][:]],
                outs=[rule_fl[1][:]],
            )
            run_phase(phases[2], rule_fl[1][:], "rule", R_SH, make_rule_done(2))
            nc.gpsimd.collective_compute(
                "AllGather",
                mybir.AluOpType.bypass,
                replica_groups=[list(range(N_CORES))],
                ins=[rule_sl[2][:]],
                outs=[rule_fl[2][:]],
            )

            # --- output phases: v2r then v2v into outbuf ---
            def p4a_done(b, pt):
                nc.vector.tensor_copy(outbuf[:, b * D : (b + 1) * D], pt[:])

            def p4b_done(b, pt):
                nc.vector.tensor_tensor(
                    out=outbuf[:, b * D : (b + 1) * D],
                    in0=outbuf[:, b * D : (b + 1) * D],
                    in1=pt[:],
                    op=mybir.AluOpType.add,
                )

            run_phase(phases[3], rule_fl[2][:], "out", O_SH, p4a_done)
            run_phase(phases[4], xb_t[:], "out", O_SH, p4b_done)

            # write outbuf -> out_slice
            for b in range(OB):
                valid = min(P, O_SH - b * P)
                nc.sync.dma_start(
                    out_t[b * P : b * P + valid, :],
                    outbuf[:valid, b * D : (b + 1) * D],
                )

    nc.compile()

    in_maps = []
    for k in range(N_CORES):
        m = {"xb": xb, "iota": iota_np}
        m["xb"] = xb
        for name, prep in [
            ("p1", ph1), ("p2", ph2), ("p3", ph3), ("p4", ph4), ("p5", ph5)
        ]:
            offs_l, rowl_l, vals_l, B, C = prep
            m[f"{name}_offs"] = offs_l[k]
            m[f"{name}_rowl"] = rowl_l[k]
            m[f"{name}_vals"] = vals_l[k]
        in_maps.append(m)

    res = run_bass_kernel_spmd(nc, in_maps, core_ids=list(range(N_CORES)))
    out = np.concatenate([res.results[k]["out_slice"] for k in range(N_CORES)], axis=0)
    return out.astype(np.float32)
